# revision 1
# baseline (speedup 1.0000x reference)
"""GroupedQueryAttention kernel for 8 Trainium2 NeuronCores.

Sharding: core c = (batch b = c//2, seq-half sh = c%2). Each core computes the
full attention output for 1024 query rows of one batch: all 8 q heads
(2 kv heads), plus the q/k/v projections and the o-projection for those rows.
Host work is limited to slicing/transposing inputs and concatenating outputs.

On-device layout: scoresT [keys, queries] so softmax-exp'd probabilities feed
attn@v matmuls directly as the moving operand (no transposes anywhere).
Denominators are produced by ones-weight matmuls replicated across all 64
output partitions, so normalization is a plain elementwise multiply.
"""

import numpy as np

B, S, D = 4, 2048, 512
H, KV, DH = 8, 2, 64
SQ = S // 2  # queries per core
NCORES = 8
PAIRS = 4  # head pairs (p, p+4); p -> kv0 rows 0:64, p+4 -> kv1 rows 64:128
SCALE = 1.0 / 8.0  # 1/sqrt(DH)
PERM = [0, 4, 1, 5, 2, 6, 3, 7]  # head order: pair-major

_built = {}


def _build_nc():
    import concourse.mybir as mybir
    import concourse.tile as tile
    from concourse import bacc

    fp32 = mybir.dt.float32
    Exp = mybir.ActivationFunctionType.Exp

    nc = bacc.Bacc("TRN2", target_bir_lowering=False, debug=False,
                   num_devices=NCORES)

    xT = nc.dram_tensor("xT", [D, S], fp32, kind="ExternalInput").ap()
    wq = nc.dram_tensor("wq", [D, D], fp32, kind="ExternalInput").ap()
    wk = nc.dram_tensor("wk", [D, KV * DH], fp32, kind="ExternalInput").ap()
    wv = nc.dram_tensor("wv", [D, KV * DH], fp32, kind="ExternalInput").ap()
    wo = nc.dram_tensor("wo", [D, D], fp32, kind="ExternalInput").ap()
    bqp = nc.dram_tensor("bqp", [128, PAIRS], fp32, kind="ExternalInput").ap()
    bkvp = nc.dram_tensor("bkvp", [128, 1], fp32, kind="ExternalInput").ap()
    bvbc = nc.dram_tensor("bvbc", [128, 128], fp32, kind="ExternalInput").ap()
    bobc = nc.dram_tensor("bobc", [128, D], fp32, kind="ExternalInput").ap()
    y = nc.dram_tensor("y", [SQ, D], fp32, kind="ExternalOutput").ap()

    with tile.TileContext(nc) as tc:
        with (
            tc.tile_pool(name="consts", bufs=1) as consts,
            tc.tile_pool(name="epool", bufs=3) as epool,
            tc.tile_pool(name="opool", bufs=6) as opool,
            tc.tile_pool(name="rpool", bufs=2) as rpool,
            tc.tile_pool(name="ypool", bufs=3) as ypool,
            tc.tile_pool(name="pssc", bufs=2, space="PSUM") as pssc,
            tc.tile_pool(name="ps512", bufs=4, space="PSUM") as ps512,
        ):
            # ---- load constants / inputs ----
            xt_sb = consts.tile([128, 4, S], fp32, tag="xt")
            nc.sync.dma_start(xt_sb[:], xT.rearrange("(c p) s -> p c s", p=128))
            wq_sb = consts.tile([128, 4, D], fp32, tag="wq")
            nc.sync.dma_start(wq_sb[:], wq.rearrange("(c p) j -> p c j", p=128))
            wk_sb = consts.tile([128, 4, 128], fp32, tag="wk")
            nc.sync.dma_start(wk_sb[:], wk.rearrange("(c p) j -> p c j", p=128))
            wv_sb = consts.tile([128, 4, 128], fp32, tag="wv")
            nc.sync.dma_start(wv_sb[:], wv.rearrange("(c p) j -> p c j", p=128))
            wo_sb = consts.tile([128, 4, D], fp32, tag="wo")
            nc.sync.dma_start(wo_sb[:], wo.rearrange("(c p) j -> p c j", p=128))
            bq_sb = consts.tile([128, PAIRS], fp32, tag="bq")
            nc.sync.dma_start(bq_sb[:], bqp)
            bkv_sb = consts.tile([128, 1], fp32, tag="bkv")
            nc.sync.dma_start(bkv_sb[:], bkvp)
            bv_sb = consts.tile([128, 128], fp32, tag="bv")
            nc.sync.dma_start(bv_sb[:], bvbc)
            bo_sb = consts.tile([128, D], fp32, tag="bo")
            nc.sync.dma_start(bo_sb[:], bobc)
            ones_sb = consts.tile([128, DH], fp32, tag="ones")
            nc.vector.memset(ones_sb[:], 1.0)

            # ---- projections ----
            # kT [128 (kv0|kv1 head-dim), S]
            kt_sb = consts.tile([128, S], fp32, tag="kt")
            for sc in range(S // 512):
                ps = ps512.tile([128, 512], fp32, tag="ps512")
                for c in range(4):
                    nc.tensor.matmul(ps[:], wk_sb[:, c, :],
                                     xt_sb[:, c, sc * 512:(sc + 1) * 512],
                                     start=(c == 0), stop=(c == 3))
                nc.vector.tensor_scalar_add(kt_sb[:, sc * 512:(sc + 1) * 512],
                                            ps[:], bkv_sb[:, 0:1])
            # v natural [s-block, 128][(kv0|kv1) head-dim]
            v_sb = consts.tile([128, S // 128, 128], fp32, tag="v")
            for sb in range(S // 128):
                ps = ps512.tile([128, 512], fp32, tag="ps512")
                for c in range(4):
                    nc.tensor.matmul(ps[:, 0:128],
                                     xt_sb[:, c, sb * 128:(sb + 1) * 128],
                                     wv_sb[:, c, :],
                                     start=(c == 0), stop=(c == 3))
                nc.vector.tensor_add(v_sb[:, sb, :], ps[:, 0:128], bv_sb[:])
            # qT [128 (head p | head p+4), SQ] per pair chunk
            qt_sb = consts.tile([128, PAIRS, SQ], fp32, tag="qt")
            for pr in range(PAIRS):
                for sc in range(SQ // 512):
                    ps = ps512.tile([128, 512], fp32, tag="ps512")
                    for c in range(4):
                        nc.tensor.matmul(ps[:],
                                         wq_sb[:, c, pr * 128:(pr + 1) * 128],
                                         xt_sb[:, c, sc * 512:(sc + 1) * 512],
                                         start=(c == 0), stop=(c == 3))
                    nc.vector.tensor_scalar_add(
                        qt_sb[:, pr, sc * 512:(sc + 1) * 512], ps[:],
                        bq_sb[:, pr:pr + 1])

            # ---- attention + o-proj ----
            NKB = S // 128  # 16 key blocks
            for qc in range(SQ // 512):
                ot_tiles = []
                for pr in range(PAIRS):
                    acc = ps512.tile([128, 512], fp32, tag="ps512")
                    den = ps512.tile([128, 512], fp32, tag="ps512")
                    e_tiles = [None] * NKB

                    def attnv(kb):
                        e = e_tiles[kb]
                        nc.tensor.matmul(acc[0:64, :], v_sb[:, kb, 0:64],
                                         e[:, 0:512],
                                         start=(kb == 0), stop=(kb == NKB - 1),
                                         tile_position=(0, 0))
                        nc.tensor.matmul(acc[64:128, :], v_sb[:, kb, 64:128],
                                         e[:, 512:1024],
                                         start=(kb == 0), stop=(kb == NKB - 1),
                                         tile_position=(0, 64))
                        nc.tensor.matmul(den[0:64, :], ones_sb[:],
                                         e[:, 0:512],
                                         start=(kb == 0), stop=(kb == NKB - 1),
                                         tile_position=(0, 0))
                        nc.tensor.matmul(den[64:128, :], ones_sb[:],
                                         e[:, 512:1024],
                                         start=(kb == 0), stop=(kb == NKB - 1),
                                         tile_position=(0, 64))

                    for kb in range(NKB):
                        sc_ps = pssc.tile([128, 1024], fp32, tag="scores")
                        nc.tensor.matmul(
                            sc_ps[:, 0:512],
                            kt_sb[0:64, kb * 128:(kb + 1) * 128],
                            qt_sb[0:64, pr, qc * 512:(qc + 1) * 512])
                        nc.tensor.matmul(
                            sc_ps[:, 512:1024],
                            kt_sb[64:128, kb * 128:(kb + 1) * 128],
                            qt_sb[64:128, pr, qc * 512:(qc + 1) * 512])
                        e = epool.tile([128, 1024], fp32, tag="E")
                        e_tiles[kb] = e
                        nc.scalar.activation(e[:], sc_ps[:], Exp, scale=SCALE)
                        # software pipeline: consume previous block's probs so
                        # PE never waits on the exp of the current block
                        if kb >= 1:
                            attnv(kb - 1)
                    attnv(NKB - 1)

                    rb = rpool.tile([128, 512], fp32, tag="recip")
                    scr = rpool.tile([128, 512], fp32, tag="rscr")
                    nc.vector.reciprocal_approx_accurate(rb[:], den[:], scr[:])
                    ot = opool.tile([128, 512], fp32, tag="outT")
                    nc.vector.tensor_mul(ot[:], acc[:], rb[:])
                    ot_tiles.append(ot)
                for m in range(4):
                    yp = ps512.tile([128, 512], fp32, tag="ps512")
                    for pr2 in range(PAIRS):
                        nc.tensor.matmul(yp[:],
                                         ot_tiles[pr2][:, m * 128:(m + 1) * 128],
                                         wo_sb[:, pr2, :],
                                         start=(pr2 == 0), stop=(pr2 == 3))
                    yt = ypool.tile([128, 512], fp32, tag="y")
                    nc.vector.tensor_add(yt[:], yp[:], bo_sb[:])
                    blk = qc * 4 + m
                    nc.sync.dma_start(y[blk * 128:(blk + 1) * 128, :], yt[:])

    nc.finalize()
    return nc


def _get_nc():
    if "nc" not in _built:
        _built["nc"] = _build_nc()
    return _built["nc"]


def kernel(x, Wq, bq, Wk, bk, Wv, bv, Wo, bo):
    from concourse.bass_utils import run_bass_kernel_spmd

    x = np.ascontiguousarray(np.asarray(x, np.float32))
    Wq = np.asarray(Wq, np.float32)
    bq = np.asarray(bq, np.float32)
    Wk = np.asarray(Wk, np.float32)
    bk = np.asarray(bk, np.float32)
    Wv = np.asarray(Wv, np.float32)
    bv = np.asarray(bv, np.float32)
    Wo = np.asarray(Wo, np.float32)
    bo = np.asarray(bo, np.float32)

    wq_p = np.ascontiguousarray(
        Wq.reshape(D, H, DH)[:, PERM, :].reshape(D, D))
    wo_p = np.ascontiguousarray(Wo.reshape(H, DH, D)[PERM].reshape(D, D))
    bq_p = np.ascontiguousarray(
        bq.reshape(H, DH)[PERM].reshape(PAIRS, 128).T)
    bkv_p = np.ascontiguousarray(bk.reshape(128, 1))
    bv_bc = np.ascontiguousarray(np.tile(bv[None, :], (128, 1)))
    bo_bc = np.ascontiguousarray(np.tile(bo[None, :], (128, 1)))

    in_maps = []
    for c in range(NCORES):
        b, sh = divmod(c, 2)
        xroll = np.roll(x[b], -sh * SQ, axis=0)
        in_maps.append({
            "xT": np.ascontiguousarray(xroll.T),
            "wq": wq_p, "wk": Wk, "wv": Wv, "wo": wo_p,
            "bqp": bq_p, "bkvp": bkv_p, "bvbc": bv_bc, "bobc": bo_bc,
        })

    nc = _get_nc()
    res = run_bass_kernel_spmd(nc, in_maps, list(range(NCORES)))
    out = np.empty((B, S, D), np.float32)
    for c in range(NCORES):
        b, sh = divmod(c, 2)
        out[b, sh * SQ:(sh + 1) * SQ, :] = res.results[c]["y"]
    return out
